# revision 2
# baseline (speedup 1.0000x reference)
"""Self-contained Trainium2 Bass kernel for the 3-layer GCN
(nn_Decoder_64020782514981): kernel(**inputs) -> np.ndarray [20000, 128] f32.

Design (v3): GCN layers are linear until the relu, so each layer is computed
as aggregate-then-transform. Layer 1 aggregates the raw bf16 x table (an
input replica on every core - zero communication), layer 2 aggregates the
communicated post-relu h1 table, and layer 3 aggregates the communicated
z3 = h2 @ W3 table (128-wide, half the bytes of h2).

Per-core work is blocks of 125 destination rows. The segment-sum runs on
TensorE: per 128-edge chunk, one matmul with the host-built one-hot
selection matrix S (edge -> dst-local, gcn norm folded into the values) as
the stationary operand and the dma_gather'ed messages as the 256-wide
moving operand. Feature-major copies for the following W-transform come
from two PE transposes per block. Cross-core distribution is chunked
AllToAll with 8x-replicated inputs (4 chunks per table, overlapped with
the aggregation pipeline) - measured ~4x faster than ring AllGather here.

Host-side prep is index plumbing only: edge bucketing, padding, one-hot
selection tables, gather indices, dtype conversion. All model FLOPs run
on device.
"""
import numpy as np
import ml_dtypes

from concourse import bass, bacc, mybir
import concourse.tile as tile

P = 128
F32 = mybir.dt.float32
BF16 = mybir.dt.bfloat16
I16 = mybir.dt.int16


class Cfg:
    def __init__(self, N, E, HID, OUT, n_cores, cpb, has_bias,
                 blk=125, agchunk=5, transport="a2a"):
        self.N, self.E, self.HID, self.OUT = N, E, HID, OUT
        self.NC = n_cores
        self.SH = N // n_cores              # nodes per core (2500)
        self.BLK = blk                      # dst rows per block (<=128)
        self.NT = self.SH // blk            # blocks per core (20)
        self.KC = HID // P                  # feature chunks (2)
        self.OC = OUT // P                  # out feature chunks (1)
        self.CPB = cpb                      # edge chunks per block
        self.G = agchunk                    # blocks per AG chunk
        self.NCH = self.NT // agchunk       # chunks per layer
        self.has_bias = has_bias
        self.transport = transport


def _prep_base(x, edge_index, W1, b1, W2, b2, W3, b3, n_cores=8,
         blk=125, agchunk=5, transport="a2a"):
    N, HID = x.shape
    OUT = W3.shape[1]
    E = edge_index.shape[1]
    SH = N // n_cores
    NT = SH // blk

    src = np.asarray(edge_index[0], dtype=np.int64)
    dst = np.asarray(edge_index[1], dtype=np.int64)

    deg = np.bincount(dst, minlength=N).astype(np.float32) + 1.0
    dinv = (1.0 / np.sqrt(deg)).astype(np.float32)

    has_bias = bool(np.any(b1) or np.any(b2) or np.any(b3))

    order = np.argsort(dst, kind="stable")
    src_s, dst_s = src[order], dst[order]

    # (core, block) buckets with self loops appended
    buckets = []
    for c in range(n_cores):
        lo = c * SH
        for b in range(NT):
            blk_lo = lo + b * blk
            blk_hi = blk_lo + blk
            i0 = np.searchsorted(dst_s, blk_lo)
            i1 = np.searchsorted(dst_s, blk_hi)
            bsrc = np.concatenate([src_s[i0:i1],
                                   np.arange(blk_lo, blk_hi, dtype=np.int64)])
            bdst = np.concatenate([dst_s[i0:i1],
                                   np.arange(blk_lo, blk_hi, dtype=np.int64)])
            bnorm = (dinv[bsrc] * dinv[bdst]).astype(np.float32)
            buckets.append((bsrc, (bdst - blk_lo).astype(np.int64), bnorm))

    cpb = max((len(bb[0]) + P - 1) // P for bb in buckets)
    cfg = Cfg(N, E, HID, OUT, n_cores, cpb, has_bias, blk, agchunk, transport)

    # chunked-AG table row remap: node (c, l) -> chunk-interleaved row
    CH = agchunk * blk  # rows per core per chunk (625)

    def remap(node):
        c, l = node // SH, node % SH
        return (l // CH) * (CH * n_cores) + c * CH + (l % CH)

    def wrap_idxs(I):
        # [cpb*128] int -> [128, cpb*8] wrapped-16 layout
        w16 = I.reshape(-1, 16).T  # [16, cpb*8]
        return np.tile(w16, (8, 1)).astype(np.int16)

    Wb_list = []
    for W, ow in ((W1, HID), (W2, HID), (W3, OUT)):
        w = np.asarray(W, np.float32).astype(ml_dtypes.bfloat16)
        # [HID, ow] -> [128, KC*ow]: W[kc*128+p, o] -> Wb[p, kc*ow + o]
        Wb_list.append(np.ascontiguousarray(
            w.reshape(HID // P, P, ow).transpose(1, 0, 2).reshape(P, -1)))

    B1c = np.zeros((P, HID // P), np.float32)  # bias per feature partition/chunk
    B2c = np.zeros((P, HID // P), np.float32)
    for Bc, b in ((B1c, b1), (B2c, b2)):
        bb = np.asarray(b, np.float32)
        Bc[:, :] = bb.reshape(HID // P, P).T
    B3r = np.tile(np.asarray(b3, np.float32), (P, 1))  # [P, OUT] replicated
    B1r = np.tile(np.asarray(b1, np.float32), (P, 1))  # [P, HID] replicated

    x_tab = np.ascontiguousarray(np.asarray(x, np.float32).astype(ml_dtypes.bfloat16))

    in_maps = []
    for c in range(n_cores):
        idxs1 = np.zeros((P, NT * cpb * 8), np.int16)
        idxs23 = np.zeros((P, NT * cpb * 8), np.int16)
        S_host = np.zeros((P, NT * cpb, P), np.float32)
        for b in range(NT):
            bsrc, bdl, bnorm = buckets[c * NT + b]
            n = len(bsrc)
            npad = cpb * P
            I1 = np.zeros(npad, np.int64)
            I1[:n] = bsrc
            idxs1[:, b * cpb * 8:(b + 1) * cpb * 8] = wrap_idxs(I1)
            idxs23[:, b * cpb * 8:(b + 1) * cpb * 8] = wrap_idxs(remap(I1))
            # S[p, b*cpb+k, j] = norm of edge slot k*128+p if dstlocal == j
            sl = np.zeros((npad, P), np.float32)
            sl[np.arange(n), bdl] = bnorm
            S_host[:, b * cpb:(b + 1) * cpb, :] = \
                sl.reshape(cpb, P, P).transpose(1, 0, 2)
        in_maps.append({
            "x_tab": x_tab,
            "idxs1": idxs1,
            "idxs23": idxs23,
            "S": S_host.astype(ml_dtypes.bfloat16).reshape(P, -1),
            "W1b": Wb_list[0], "W2b": Wb_list[1], "W3b": Wb_list[2],
            "B1c": B1c, "B2c": B2c, "B3r": B3r, "B1r": B1r,
        })
    return cfg, in_maps



def build(cfg: Cfg) -> bass.Bass:
    N, HID, OUT = cfg.N, cfg.HID, cfg.OUT
    SH, NT, KC, CPB, BLK = cfg.SH, cfg.NT, cfg.KC, cfg.CPB, cfg.BLK
    G, NCH = cfg.G, cfg.NCH
    CH = G * BLK

    nc = bacc.Bacc(None, target_bir_lowering=False, num_devices=cfg.NC,
                   num_swdge_queues=4)

    x_tab_in = nc.declare_dram_parameter("x_tab", [N, HID], BF16, isOutput=False)
    idxs1_in = nc.declare_dram_parameter("idxs1", [P, NT * CPB * 8], I16, isOutput=False)
    idxs23_in = nc.declare_dram_parameter("idxs23", [P, NT * CPB * 8], I16, isOutput=False)
    S_in = nc.declare_dram_parameter("S", [P, NT * CPB * P], BF16, isOutput=False)
    W1_in = nc.declare_dram_parameter("W1b", [P, KC * HID], BF16, isOutput=False)
    W2_in = nc.declare_dram_parameter("W2b", [P, KC * HID], BF16, isOutput=False)
    W3_in = nc.declare_dram_parameter("W3b", [P, KC * OUT], BF16, isOutput=False)
    B1_in = nc.declare_dram_parameter("B1c", [P, KC], F32, isOutput=False)
    B2_in = nc.declare_dram_parameter("B2c", [P, KC], F32, isOutput=False)
    B3_in = nc.declare_dram_parameter("B3r", [P, OUT], F32, isOutput=False)
    B1r_in = nc.declare_dram_parameter("B1r", [P, HID], F32, isOutput=False)
    ident_in = nc.declare_dram_parameter("ident", [P, P], BF16, isOutput=False)
    out_ext = nc.declare_dram_parameter("out", [SH, OUT], F32, isOutput=True)

    NC8 = cfg.NC
    a2a = cfg.transport == "a2a"
    if a2a:
        h1_loc = [nc.dram_tensor(f"h1rep{g}", [NC8 * CH, HID], BF16)
                  for g in range(NCH)]
        z3_loc = [nc.dram_tensor(f"z3rep{g}", [NC8 * CH, OUT], BF16)
                  for g in range(NCH)]
        h1_tab = nc.dram_tensor("h1tab", [N, HID], BF16)
        z3_tab = nc.dram_tensor("z3tab", [N, OUT], BF16)
    else:
        h1_loc = [nc.dram_tensor(f"h1loc{g}", [CH, HID], BF16) for g in range(NCH)]
        z3_loc = [nc.dram_tensor(f"z3loc{g}", [CH, OUT], BF16) for g in range(NCH)]
        h1_tab = nc.dram_tensor("h1tab", [N, HID], BF16, addr_space="Shared")
        z3_tab = nc.dram_tensor("z3tab", [N, OUT], BF16, addr_space="Shared")

    core_ids = list(range(cfg.NC))
    nc.gpsimd.bir_kernel_barrier_wait([core_ids])

    with tile.TileContext(nc) as tc:
        with (
            tc.tile_pool(name="persist", bufs=1) as pp,
            tc.tile_pool(name="msg", bufs=4) as msg_pool,
            tc.tile_pool(name="an", bufs=3) as an_pool,
            tc.tile_pool(name="at", bufs=3) as at_pool,
            tc.tile_pool(name="hsb", bufs=3) as hsb_pool,
            tc.tile_pool(name="ht", bufs=3) as ht_pool,
            tc.tile_pool(name="osb", bufs=3) as osb_pool,
            tc.tile_pool(name="psa", bufs=2, space="PSUM") as psa_pool,
            tc.tile_pool(name="pst", bufs=2, space="PSUM") as pst_pool,
            tc.tile_pool(name="psh", bufs=2, space="PSUM") as psh_pool,
            tc.tile_pool(name="psz", bufs=2, space="PSUM") as psz_pool,
        ):
            S_sb = pp.tile([P, NT * CPB, P], BF16, tag="S")
            idxs1_sb = pp.tile([P, NT * CPB * 8], I16, tag="idxs1")
            idxs23_sb = pp.tile([P, NT * CPB * 8], I16, tag="idxs23")
            W1_sb = pp.tile([P, KC, HID], BF16, tag="w1")
            W2_sb = pp.tile([P, KC, HID], BF16, tag="w2")
            W3_sb = pp.tile([P, KC, OUT], BF16, tag="w3")
            B1_sb = pp.tile([P, KC], F32, tag="b1")
            B2_sb = pp.tile([P, KC], F32, tag="b2")
            B3_sb = pp.tile([P, OUT], F32, tag="b3")
            B1r_sb = pp.tile([P, HID], F32, tag="b1r")
            ident_sb = pp.tile([P, P], BF16, tag="ident")

            nc.sync.dma_start(out=idxs1_sb[:], in_=idxs1_in[:])
            nc.sync.dma_start(out=idxs23_sb[:], in_=idxs23_in[:])
            nc.sync.dma_start(out=W1_sb[:], in_=W1_in[:].rearrange("p (c o) -> p c o", c=KC))
            nc.sync.dma_start(out=W2_sb[:], in_=W2_in[:].rearrange("p (c o) -> p c o", c=KC))
            nc.sync.dma_start(out=W3_sb[:], in_=W3_in[:].rearrange("p (c o) -> p c o", c=KC))
            nc.sync.dma_start(out=ident_sb[:], in_=ident_in[:])
            if cfg.has_bias:
                nc.sync.dma_start(out=B1_sb[:], in_=B1_in[:])
                nc.sync.dma_start(out=B2_sb[:], in_=B2_in[:])
                nc.sync.dma_start(out=B3_sb[:], in_=B3_in[:])
                nc.sync.dma_start(out=B1r_sb[:], in_=B1r_in[:])
            Scols = NT * CPB * P
            for g in range(NCH):
                c0 = g * (Scols // NCH)
                c1 = (g + 1) * (Scols // NCH)
                nc.sync.dma_start(
                    out=S_sb[:].rearrange("p k j -> p (k j)")[:, c0:c1],
                    in_=S_in[:, c0:c1])

            r_nidx = nc.gpsimd.to_reg(CPB * P)

            def distribute(loc, tab, g):
                if a2a:
                    for j in range(1, NC8):
                        nc.sync.dma_start(
                            out=loc[g][j * CH:(j + 1) * CH, :],
                            in_=loc[g][0:CH, :])
                    nc.gpsimd.collective_compute(
                        "AllToAll", mybir.AluOpType.bypass,
                        ins=[loc[g][:].opt()],
                        outs=[tab[g * CH * NC8:(g + 1) * CH * NC8, :].opt()],
                        replica_groups=[core_ids])
                else:
                    nc.gpsimd.collective_compute(
                        "AllGather", mybir.AluOpType.bypass,
                        ins=[loc[g][:].opt()],
                        outs=[tab[g * CH * NC8:(g + 1) * CH * NC8, :].opt()],
                        replica_groups=[core_ids])

            def gather(tab, idxs_sb, b, width, q):
                msg = msg_pool.tile([P, CPB, width], BF16,
                                    tag="msg" if width == HID else "msg3")
                nc.gpsimd.dma_gather(
                    out_ap=msg[:], in_ap=tab[:],
                    idxs_ap=idxs_sb[:, b * CPB * 8:(b + 1) * CPB * 8],
                    num_idxs=CPB * P, num_idxs_reg=r_nidx,
                    elem_size=width, single_packet=False,
                    queue_num=q)
                return msg

            def agg_nm_aT(b, msg):
                """node-major agg + transposed bf16 copy: (a_node, aT)."""
                pA = psa_pool.tile([P, HID], F32, tag="pa")
                for k in range(CPB):
                    nc.tensor.matmul(
                        out=pA[:BLK, :],
                        lhsT=S_sb[:, b * CPB + k, :BLK],
                        rhs=msg[:, k, :],
                        start=(k == 0), stop=(k == CPB - 1))
                a_node = an_pool.tile([P, HID], BF16, tag="an")
                nc.scalar.activation(
                    out=a_node[:BLK, :], in_=pA[:BLK, :],
                    func=mybir.ActivationFunctionType.Copy)
                aT = at_pool.tile([P, KC, BLK], BF16, tag="at")
                for fh in range(KC):
                    pT = pst_pool.tile([P, P], BF16, tag="pt")
                    nc.tensor.transpose(
                        out=pT[:, :BLK],
                        in_=a_node[:BLK, fh * P:(fh + 1) * P],
                        identity=ident_sb[:BLK, :BLK])
                    nc.scalar.activation(
                        out=aT[:, fh, :], in_=pT[:, :BLK],
                        func=mybir.ActivationFunctionType.Copy)
                return aT

            # =========== phase A: layer 1 ===========
            for b in range(NT):
                g, brel = b // G, b % G
                msg = gather(x_tab_in, idxs1_sb, b, HID, b % 4)
                aT = agg_nm_aT(b, msg)
                pH = psh_pool.tile([P, HID], F32, tag="ph")
                for kc in range(KC):
                    nc.tensor.matmul(
                        out=pH[:BLK, :], lhsT=aT[:, kc, :],
                        rhs=W1_sb[:, kc, :],
                        start=(kc == 0), stop=(kc == KC - 1))
                h_sb = hsb_pool.tile([P, HID], BF16, tag="hsb")
                if cfg.has_bias:
                    nc.vector.tensor_add(out=pH[:BLK, :], in0=pH[:BLK, :],
                                         in1=B1r_sb[:BLK, :])
                nc.scalar.activation(
                    out=h_sb[:BLK, :], in_=pH[:BLK, :],
                    func=mybir.ActivationFunctionType.Relu)
                nc.sync.dma_start(out=h1_loc[g][brel * BLK:(brel + 1) * BLK, :],
                                  in_=h_sb[:BLK, :])
                if brel == G - 1:
                    distribute(h1_loc, h1_tab, g)

            # =========== phase B: layer 2 + z3 ===========
            for b in range(NT):
                g, brel = b // G, b % G
                msg = gather(h1_tab, idxs23_sb, b, HID, b % 4)
                aT = agg_nm_aT(b, msg)
                hT = ht_pool.tile([P, KC, BLK], BF16, tag="ht")
                for fo in range(KC):
                    pT = pst_pool.tile([P, P], F32, tag="pt")
                    for kc in range(KC):
                        nc.tensor.matmul(
                            out=pT[:, :BLK],
                            lhsT=W2_sb[:, kc, fo * P:(fo + 1) * P],
                            rhs=aT[:, kc, :],
                            start=(kc == 0), stop=(kc == KC - 1))
                    nc.scalar.activation(
                        out=hT[:, fo, :], in_=pT[:, :BLK],
                        func=mybir.ActivationFunctionType.Relu)
                pz = psz_pool.tile([P, OUT], F32, tag="pz")
                for kc in range(KC):
                    nc.tensor.matmul(
                        out=pz[:BLK, :], lhsT=hT[:, kc, :],
                        rhs=W3_sb[:, kc, :],
                        start=(kc == 0), stop=(kc == KC - 1))
                z_sb = hsb_pool.tile([P, OUT], BF16, tag="zsb")
                nc.scalar.activation(
                    out=z_sb[:BLK, :], in_=pz[:BLK, :],
                    func=mybir.ActivationFunctionType.Copy)
                nc.sync.dma_start(out=z3_loc[g][brel * BLK:(brel + 1) * BLK, :],
                                  in_=z_sb[:BLK, :])
                if brel == G - 1:
                    distribute(z3_loc, z3_tab, g)

            # =========== phase C: layer 3 ===========
            for b in range(NT):
                msg = gather(z3_tab, idxs23_sb, b, OUT, b % 4)
                pO = psz_pool.tile([P, OUT], F32, tag="pz")
                for k in range(CPB):
                    nc.tensor.matmul(
                        out=pO[:BLK, :],
                        lhsT=S_sb[:, b * CPB + k, :BLK],
                        rhs=msg[:, k, :],
                        start=(k == 0), stop=(k == CPB - 1))
                o_sb = osb_pool.tile([P, OUT], F32, tag="osb")
                if cfg.has_bias:
                    nc.vector.tensor_add(out=o_sb[:BLK, :], in0=pO[:BLK, :],
                                         in1=B3_sb[:BLK, :])
                else:
                    nc.scalar.activation(
                        out=o_sb[:BLK, :], in_=pO[:BLK, :],
                        func=mybir.ActivationFunctionType.Copy)
                nc.sync.dma_start(out=out_ext[b * BLK:(b + 1) * BLK, :],
                                  in_=o_sb[:BLK, :])

    nc.finalize()
    split_sync_waits(nc)
    return nc



_counter = [0]


def split_sync_waits(nc, maxw=1):
    n_split = 0
    for f in nc.m.functions:
        for bb in f.blocks:
            insts = list(bb.instructions)
            out = []
            changed = False
            for inst in insts:
                si = inst.sync_info
                if si is not None and len(si.on_wait) > maxw:
                    waits = list(si.on_wait)
                    keep = waits[-maxw:] if maxw else []
                    rest = waits[: len(waits) - maxw]
                    for w in rest:
                        _counter[0] += 1
                        nop = mybir.InstNoOp(
                            name=f"wspill-{_counter[0]}",
                            engine=inst.engine,
                            bass_nofuse=True,
                            sync_info=mybir.SyncInfo(on_wait=[w], on_update=[]),
                        )
                        nc.register_instruction(nop)
                        out.append(nop)
                    si.on_wait = keep
                    changed = True
                    n_split += 1
                out.append(inst)
            if changed:
                bb.instructions = out
    return n_split




def prep(x, edge_index, W1, b1, W2, b2, W3, b3, n_cores=8,
         blk=125, agchunk=5, transport="a2a"):
    cfg, in_maps = _prep_base(x, edge_index, W1, b1, W2, b2, W3, b3,
                              n_cores=n_cores, blk=blk, agchunk=agchunk,
                              transport=transport)
    eye = np.eye(P, dtype=ml_dtypes.bfloat16)
    for m in in_maps:
        m["ident"] = eye
    return cfg, in_maps


def kernel(**inputs):
    from concourse.bass_utils import run_bass_kernel_spmd

    cfg, in_maps = prep(
        np.asarray(inputs["x"], np.float32), np.asarray(inputs["edge_index"]),
        np.asarray(inputs["W1"], np.float32), np.asarray(inputs["b1"], np.float32),
        np.asarray(inputs["W2"], np.float32), np.asarray(inputs["b2"], np.float32),
        np.asarray(inputs["W3"], np.float32), np.asarray(inputs["b3"], np.float32))
    nc = build(cfg)
    res = run_bass_kernel_spmd(nc, in_maps, core_ids=list(range(cfg.NC)))
    out = np.concatenate([res.results[c]["out"] for c in range(cfg.NC)], axis=0)
    return out.astype(np.float32)
